# revision 35
# baseline (speedup 1.0000x reference)
"""Trainium2 Bass kernel for nn_ConvSelfAttention.

Math: the reference computes, per head h,
    kv   = conv3x3(x, w_kv[h]) + b_kv[h]                     # [B*T,19,19,16]
    q    = conv3x3(x, w_q[h])  + b_q[h]
    att[b,tq,tk] = conv3x3(concat[kv[tk], q[tq]], w_att[h]) + b_att[h]
                 = A_k[b,tk] + A_q[b,tq] + b_att[h]          # conv is linear in channels
    soft = softmax_tk(att)                                   # additive tq-terms cancel
         = softmax_tk(A_k[b,tk])
    out[b,tq] = sum_tk kv[b,tk] * soft[b,tk]                 # independent of tq!
So the q path (w_q, b_q) and b_att never affect the output, and the result
broadcasts over the query-time axis.

Matmul cost on PE = out free-size (N) x cycles/row, independent of K and M.
So every conv matmul packs the full K=128 contraction:
  stage A (kv conv): K = 64 ch x 2 images (block-diagonal weights); image
    pair (2m, 2m+1) lives at SBUF partitions 0-63 / 64-127 at the same
    column offset.  9 taps x 16 pairs = 144 matmuls (vs 288 at K=64).
  stage B (score conv): K = 32 kv-ch x 4 images of one quad (kvb[g] is
    already [4 img x 32 ch, 441]); block-diag weights map img j, head h to
    out row 2j+h.  9 taps x 8 quads = 72 matmuls (vs 288 at K=32/64).
Scores land sparse (8 rows per 32-partition block, tile_position col must be
32-aligned); the exp() after the PE transpose compacts them via a strided AP.
"""

import sys

import ml_dtypes
import numpy as np

if "/opt/trn_rl_repo" not in sys.path:
    sys.path.insert(0, "/opt/trn_rl_repo")

import concourse.bass as bass
import concourse.mybir as mybir
import concourse.tile as tile
from concourse import bacc
from concourse.bass_utils import run_bass_kernel_spmd

# problem constants (hardcoded per contract)
B, T, HS, WS, C, NH = 4, 32, 19, 19, 64, 4
D = C // NH            # 16 per-head channels
PX = HS * WS           # 361 pixels
NCORE = 8
HPC = 2                # heads per core
M32 = HPC * D          # 32 kv channels per core
NQUAD = 8              # image quads (4 imgs each)
XCOLS = 16 * 361       # x_sb cols: 16 unpadded image-pair blocks
CHUNKS = [(0, 128), (128, 128), (256, 105)]  # pixel chunks (start, count)
# tap = 3*dy + dx; center tap first so every psum element is written before
# other taps accumulate onto it (per-element has_written semantics)
TAP_ORDER = [4, 0, 1, 2, 3, 5, 6, 7, 8]


def _tap_rects(tap):
    """Valid output rect and matching input offset for a SAME-pad conv tap."""
    dy, dx = tap // 3, tap % 3
    oy0, oy1 = max(0, 1 - dy), HS - max(0, dy - 1)
    ox0, ox1 = max(0, 1 - dx), WS - max(0, dx - 1)
    iy0, ix0 = oy0 + dy - 1, ox0 + dx - 1
    return oy0, ox0, oy1 - oy0, ox1 - ox0, iy0, ix0

F32 = mybir.dt.float32
BF16 = mybir.dt.bfloat16


def _kernel_body(tc, y, x_t, w_kv_t, w_s_t, b_vec, ident):
    nc = tc.nc

    from contextlib import ExitStack

    with ExitStack() as ctx:
        const = ctx.enter_context(tc.tile_pool(name="const", bufs=1))
        kvpool = ctx.enter_context(tc.tile_pool(name="kv", bufs=1))
        sbig = ctx.enter_context(tc.tile_pool(name="sbig", bufs=1))
        small = ctx.enter_context(tc.tile_pool(name="small", bufs=1))
        tmppool = ctx.enter_context(tc.tile_pool(name="tmp", bufs=2))
        psA = ctx.enter_context(tc.tile_pool(name="psA", bufs=2, space="PSUM"))
        psS = ctx.enter_context(tc.tile_pool(name="psS", bufs=1, space="PSUM"))
        psK = ctx.enter_context(tc.tile_pool(name="psK", bufs=2, space="PSUM"))
        psT = ctx.enter_context(tc.tile_pool(name="psT", bufs=2, space="PSUM"))

        # ---- load inputs: one packed bf16 const DMA (HWDGE costs ~625ns
        # per dma_start, serialized), then x in 3 chunks so stage A starts
        # as soon as quad 0 lands
        wpack_sb = const.tile([128, 777], BF16)
        nc.sync.dma_start(wpack_sb[:], w_kv_t[:])
        x_sb = const.tile([128, XCOLS], BF16)
        nc.sync.dma_start(x_sb[:, 0:722], x_t[:, 0:722])
        id_sb = const.tile([128, 128], F32)
        nc.sync.dma_start(id_sb[:], ident[:])
        nc.sync.dma_start(x_sb[:, 722 : 4 * 722], x_t[:, 722 : 4 * 722])
        nc.sync.dma_start(x_sb[:, 4 * 722 :], x_t[:, 4 * 722 :])
        # bias must be f32 for tensor_scalar ops: tiny on-device convert
        b32_sb = const.tile([128, 1], F32)
        nc.scalar.copy(b32_sb[:], wpack_sb[:, 648:649])
        # warm the Exp activation table off the critical path
        warm = const.tile([1, 1], F32)
        nc.gpsimd.memset(warm[:], 0.0)
        nc.scalar.activation(warm[:], warm[:],
                             mybir.ActivationFunctionType.Exp)
        # warm the PE p-state during the x DMA wait: pe_busy_start is set at
        # the first matmul, and full clock needs ~3us of ramp
        wz = const.tile([128, 256], BF16)
        nc.gpsimd.memset(wz[:], 0.0)
        for i in range(26):
            wps = psT.tile([128, 128], F32, tag="psTs", name=f"warm{i}")
            nc.tensor.matmul(wps[:], wz[:, 0:128], wz[:, 128:256])

        kv = [kvpool.tile([128, PX], BF16, tag=f"kv{g}", name=f"kv{g}")
              for g in range(NQUAD)]
        # padded (21x21) bf16 kv for stage-B windowed rhs reads
        kvb = [kvpool.tile([128, 441], BF16, tag=f"kvb{g}", name=f"kvb{g}")
               for g in range(NQUAD)]
        # scores, sparse rows: s_ps[half] partition 32*(g%4) + 2j + h
        s_ps = [psS.tile([128, PX], F32, tag=f"sps{hf}", name=f"sps{hf}")
                for hf in range(2)]
        s_sb = [sbig.tile([128, PX], F32, tag=f"ssb{hf}", name=f"ssb{hf}")
                for hf in range(2)]
        for hf in range(2):  # garbage rows flow through the PE transpose
            nc.gpsimd.memset(s_sb[hf][:], 0.0)
        # pixel-major kv: col = 1536*hf + 768*h + 256*c + 64*gq + 16*j + d
        kvT = sbig.tile([128, 3072], BF16)
        # exp(scores), pixel-major: col = 96*h + 48*hf + 16*c + 4*gq + j
        p_T = sbig.tile([128, 192], F32)

        def stage_a(g):
            ps = psA.tile([128, PX], F32, tag="psA", name=f"psA{g}")
            for ti, tap in enumerate(TAP_ORDER):
                oy0, ox0, nr, ncol, iy0, ix0 = _tap_rects(tap)
                lhs = wpack_sb[:, tap * 64 : (tap + 1) * 64]
                for pair in range(2):
                    m = 2 * g + pair
                    rhs = x_sb[:, m * PX : (m + 1) * PX].rearrange(
                        "p (a b) -> p a b", a=HS)[
                        :, iy0 : iy0 + nr, ix0 : ix0 + ncol]
                    out = ps[64 * pair : 64 * pair + 64, :].rearrange(
                        "p (a b) -> p a b", a=HS)[
                        :, oy0 : oy0 + nr, ox0 : ox0 + ncol]
                    nc.tensor.matmul(
                        out, lhs, rhs,
                        start=(ti == 0), stop=(ti == 8),
                        tile_position=(0, 64 * pair),
                        skip_group_check=True,
                    )
            return ps

        def stage_a_evac(g, ps):
            # evacuate: kv (+ per-channel bias) bf16, and bf16 padded copy.
            # Emitted AFTER stage_b(g-1) so the in-order DVE queue runs
            # compact(g-1) before this add (no head-of-line block).
            nc.vector.tensor_scalar_add(kv[g][:], ps[:], b32_sb[:])
            kvb_in = kvb[g][:].rearrange("p (a b) -> p a b", a=21)[:, 1:20, 1:20]
            kvsrc = kv[g][:].rearrange("p (a b) -> p a b", a=HS)
            if g == NQUAD - 1:
                nc.scalar.copy(kvb_in[:, 0:10, :], kvsrc[:, 0:10, :])
                nc.gpsimd.tensor_copy(kvb_in[:, 10:HS, :], kvsrc[:, 10:HS, :])
            else:
                nc.gpsimd.tensor_copy(kvb_in, kvsrc)

        def stage_b(g):
            hf, q = g // 4, g % 4
            for ti, tap in enumerate(TAP_ORDER):
                oy0, ox0, nr, ncol, iy0, ix0 = _tap_rects(tap)
                lhs = wpack_sb[:, 576 + tap * 8 : 576 + (tap + 1) * 8]
                rhs = kvb[g][:].rearrange("p (a b) -> p a b", a=21)[
                    :, 1 + iy0 : 1 + iy0 + nr, 1 + ix0 : 1 + ix0 + ncol]
                out = s_ps[hf][32 * q : 32 * q + 8, :].rearrange(
                    "p (a b) -> p a b", a=HS)[
                    :, oy0 : oy0 + nr, ox0 : ox0 + ncol]
                nc.tensor.matmul(
                    out, lhs, rhs,
                    start=(ti == 0), stop=(ti == 8),
                    tile_position=(0, 32 * q),
                    skip_group_check=True,
                )

        def stage_t(g):
            # kv[g] [128, px] -> pixel-major via PE transpose (bf16, 1 cyc/row)
            hf, gq = g // 4, g % 4
            ps = psK.tile([128, 384], BF16, tag="psK", name=f"psK{g}")
            for c, (p0, cnt) in enumerate(CHUNKS):
                nc.tensor.matmul(ps[0:cnt, 128 * c : 128 * c + 128],
                                 kv[g][:, p0 : p0 + cnt], wpack_sb[:, 649:777],
                                 is_transpose=True)
            for c, (p0, cnt) in enumerate(CHUNKS):
                # psK col = 32j + 16h + d  ->  kvT col 1536hf+768h+256c+64gq+16j+d
                src = ps[0:cnt, 128 * c : 128 * c + 128].rearrange(
                    "p (j h d) -> p j h d", j=4, h=2)
                ks = kvT[0:cnt, 1536 * hf + 256 * c + 64 * gq :]
                dst = bass.AP(tensor=ks.tensor, offset=ks.offset,
                              ap=[ks.ap[0], [16, 4], [768, 2], [1, D]])
                nc.scalar.copy(dst, src)

        def compact_half(hf):
            # split DVE + ACT (gpsimd cannot read PSUM)
            nc.vector.tensor_copy(s_sb[hf][:, 0:180], s_ps[hf][:, 0:180])
            nc.scalar.copy(s_sb[hf][:, 180:PX], s_ps[hf][:, 180:PX])

        def stage_s(hf):
            # transpose this half's scores, exp-compact into p_T
            for c, (p0, cnt) in enumerate(CHUNKS):
                if hf == 1 and c == 2:
                    big = psA.tile([128, PX], F32, tag="psA", name=f"psTs{c}{hf}")
                    ps_t = big[:, 0:128]
                else:
                    ps_t = psT.tile([128, 128], F32, tag="psTs",
                                    name=f"psTs{c}{hf}")
                nc.tensor.matmul(ps_t[0:cnt, :], s_sb[hf][:, p0 : p0 + cnt],
                                 id_sb, is_transpose=True)
                # ps_t col = 32q + 2j + h (cols 8-31 of each 32-block unused)
                pss = ps_t[0:cnt, :]
                src = bass.AP(tensor=pss.tensor, offset=pss.offset,
                              ap=[pss.ap[0], [32, 4], [2, 4], [1, 2]])
                pts = p_T[0:cnt, 48 * hf + 16 * c :]
                dst = bass.AP(tensor=pts.tensor, offset=pts.offset,
                              ap=[pts.ap[0], [4, 4], [1, 4], [96, 2]])
                nc.scalar.activation(dst, src, mybir.ActivationFunctionType.Exp)

        z4 = small.tile([128, 12], F32)    # cols: 6*hf + 3*h + c
        z2 = small.tile([128, 6], F32)     # cols: 3*h + c
        zi = small.tile([128, 6], F32)
        acc = small.tile([128, 192], F32)  # cols: 96*hf + 32*c + 16*h + d

        def _zchain(hf):
            # z[p, h, c] = sum over r=(gq,j) of exp-scores for this half
            pz = p_T[:, 48 * hf : 48 * hf + 48]
            zin = bass.AP(tensor=pz.tensor, offset=pz.offset,
                          ap=[pz.ap[0], [96, 2], [16, 3], [1, 16]])
            nc.vector.reduce_sum(
                z4[:, 6 * hf : 6 * hf + 6].rearrange("p (h c) -> p h c", h=2),
                zin, axis=mybir.AxisListType.X)
            if hf == 1:
                nc.vector.tensor_add(z2[:], z4[:, 0:6], z4[:, 6:12])
                nc.vector.reciprocal(zi[:], z2[:])

        def _reduce_c(hf, t, c):
            tin = t[:, 256 * c : 256 * c + 256]
            rin = bass.AP(tensor=tin.tensor, offset=tin.offset,
                          ap=[tin.ap[0], [768, 2], [1, D], [16, 16]])
            nc.vector.reduce_sum(
                acc[:, 96 * hf + 32 * c : 96 * hf + 32 * c + 32].rearrange(
                    "p (h d) -> p h d", h=2),
                rin, axis=mybir.AxisListType.X)

        def stage_d_half(hf):
            # per chunk: 2 mults (DVE+Pool) then the reduce, so the DVE
            # starts right after the first exp; z-chain last
            t = small.tile([128, 1536], F32, tag=f"tD{hf}", name=f"tD{hf}")
            for c in range(3):
                for h in range(HPC):
                    base = 1536 * hf + 768 * h + 256 * c
                    v0 = kvT[:, base : base + 256].rearrange(
                        "p (gj d) -> p gj d", d=D)
                    pv = p_T[:, 96 * h + 48 * hf + 16 * c :]
                    v1 = bass.AP(tensor=pv.tensor, offset=pv.offset,
                                 ap=[pv.ap[0], [1, 16], [0, D]])
                    eng = nc.vector if h == 0 else nc.gpsimd
                    eng.tensor_mul(
                        t[:, 768 * h + 256 * c : 768 * h + 256 * c + 256
                          ].rearrange("p (gj d) -> p gj d", d=D), v0, v1)
                _reduce_c(hf, t, c)
            _zchain(hf)
            if hf == 1:
                accS = small.tile([128, 96], F32)  # cols: 32*c + 16*h + d
                nc.vector.tensor_add(accS[:], acc[:, 0:96], acc[:, 96:192])
                outT = small.tile([128, 96], F32)
                v0 = accS[:].rearrange("p (c h d) -> p c h d", c=3, h=2)
                v1 = bass.AP(tensor=zi.tensor, offset=zi[:].offset,
                             ap=[zi[:].ap[0], [1, 3], [3, 2], [0, D]])
                nc.vector.tensor_mul(
                    outT[:].rearrange("p (c h d) -> p c h d", c=3, h=2),
                    v0, v1)
                nc.sync.dma_start(y[:], outT[:])

        def stage_d_tail():
            pass

        # software pipeline: PE order A0, A1, [T0 B0], A2, [T1 B1], ...
        pend = None
        for g in range(NQUAD):
            ps = stage_a(g)
            if g >= 1:
                stage_t(g - 1)
                stage_b(g - 1)
            if g >= 1 and g - 1 == 3:
                compact_half(0)
            stage_a_evac(g, ps)
            if g >= 1 and g - 1 == 4:
                stage_s(0)
                stage_d_half(0)
        stage_t(NQUAD - 1)
        stage_b(NQUAD - 1)
        compact_half(1)
        stage_s(1)
        stage_d_half(1)
        stage_d_tail()


_CACHE = {}


def _build_program():
    if "nc" in _CACHE:
        return _CACHE["nc"]
    nc = bacc.Bacc("TRN2", target_bir_lowering=False, debug=False,
                   num_devices=NCORE)
    x_t = nc.dram_tensor("x_t", [128, XCOLS], BF16, kind="ExternalInput").ap()
    w_kv_t = nc.dram_tensor("w_kv_t", [128, 777], BF16,
                            kind="ExternalInput").ap()
    ident = nc.dram_tensor("ident", [128, 128], F32, kind="ExternalInput").ap()
    y = nc.dram_tensor("y", [128, 96], F32, kind="ExternalOutput").ap()
    with tile.TileContext(nc) as tc:
        _kernel_body(tc, y, x_t, w_kv_t, None, None, ident)
    nc.compile()
    _CACHE["nc"] = nc
    return nc


def make_in_maps(x, w_kv, b_kv, w_att):
    """Host-side shard prep: per-core input dicts."""
    x = np.asarray(x, np.float32)
    w_kv = np.asarray(w_kv, np.float32)
    b_kv = np.asarray(b_kv, np.float32)
    w_att = np.asarray(w_att, np.float32)
    ident = np.eye(128, dtype=np.float32)
    in_maps = []
    # x_sb layout: col block m (441 = padded 21x21) holds img 2m on
    # partitions 0-63 (its 64 channels) and img 2m+1 on partitions 64-127.
    xt_all = []
    for b in range(B):
        xp = np.zeros((2, 64, 16, HS, WS), np.float32)
        for m in range(16):
            xp[0, :, m] = x[b, 2 * m].transpose(2, 0, 1)
            xp[1, :, m] = x[b, 2 * m + 1].transpose(2, 0, 1)
        xt_all.append(xp.reshape(128, XCOLS).astype(ml_dtypes.bfloat16))
    for core in range(NCORE):
        b, hb = core // 2, (core % 2) * HPC
        # stage A weights: per tap [128, 64] block-diagonal over the img pair
        wk = np.zeros((128, 9, 64), np.float32)
        for tap in range(9):
            dy, dx = tap // 3, tap % 3
            for hh in range(HPC):
                blk = w_kv[hb + hh, dy, dx]          # [64, 16]
                wk[0:64, tap, 16 * hh : 16 * hh + 16] = blk
                wk[64:128, tap, 32 + 16 * hh : 32 + 16 * hh + 16] = blk
        # stage B weights: per tap [128, 8]; row 32j+16h+d -> out col 2j+h
        ws = np.zeros((128, 9, 8), np.float32)
        for tap in range(9):
            dy, dx = tap // 3, tap % 3
            for j in range(4):
                for hh in range(HPC):
                    ws[32 * j + 16 * hh : 32 * j + 16 * hh + 16, tap,
                       2 * j + hh] = w_att[hb + hh, dy, dx, :D, 0]
        bv = np.tile(np.concatenate([b_kv[hb], b_kv[hb + 1]]), 4)
        wpack = np.concatenate(
            [wk.reshape(128, 9 * 64), ws.reshape(128, 9 * 8),
             bv.reshape(128, 1), ident], axis=1)
        in_maps.append({"x_t": xt_all[b],
                        "w_kv_t": wpack.astype(ml_dtypes.bfloat16),
                        "ident": ident})
    return in_maps


def assemble(results):
    out = np.empty((B, T, HS, WS, C), np.float32)
    for core in range(NCORE):
        b, hb = core // 2, (core % 2) * M32
        yc = np.asarray(results[core]["y"])  # [128, 32c + 16h + d]
        ypx = np.empty((PX, M32), np.float32)
        for c, (p0, cnt) in enumerate(CHUNKS):
            ypx[p0 : p0 + cnt, :] = yc[0:cnt, 32 * c : 32 * c + 32]
        out[b, :, :, :, hb : hb + M32] = ypx.reshape(HS, WS, M32)[None]
    return out


def kernel(x, w_q, b_q, w_kv, b_kv, w_att, b_att, **_unused):
    nc = _build_program()
    in_maps = make_in_maps(x, w_kv, b_kv, w_att)
    res = run_bass_kernel_spmd(nc, in_maps, core_ids=list(range(NCORE)))
    return assemble(res.results)


if __name__ == "__main__":
    rng = np.random.default_rng(0)
    ins = {
        "x": rng.standard_normal((B, T, HS, WS, C)).astype(np.float32),
        "w_q": rng.standard_normal((NH, 3, 3, C, D)).astype(np.float32) * 0.05,
        "b_q": np.zeros((NH, D), np.float32),
        "w_kv": rng.standard_normal((NH, 3, 3, C, D)).astype(np.float32) * 0.05,
        "b_kv": np.zeros((NH, D), np.float32),
        "w_att": rng.standard_normal((NH, 3, 3, 2 * D, 1)).astype(np.float32) * 0.05,
        "b_att": np.zeros((NH, 1), np.float32),
    }
    out = kernel(**ins)
    print("kernel output", out.shape, out.dtype)


# revision 39
# speedup vs baseline: 1.0061x; 1.0061x over previous
"""Trainium2 Bass kernel for nn_ConvSelfAttention.

Math: the reference computes, per head h,
    kv   = conv3x3(x, w_kv[h]) + b_kv[h]                     # [B*T,19,19,16]
    q    = conv3x3(x, w_q[h])  + b_q[h]
    att[b,tq,tk] = conv3x3(concat[kv[tk], q[tq]], w_att[h]) + b_att[h]
                 = A_k[b,tk] + A_q[b,tq] + b_att[h]          # conv is linear in channels
    soft = softmax_tk(att)                                   # additive tq-terms cancel
         = softmax_tk(A_k[b,tk])
    out[b,tq] = sum_tk kv[b,tk] * soft[b,tk]                 # independent of tq!
So the q path (w_q, b_q) and b_att never affect the output, and the result
broadcasts over the query-time axis.

Matmul cost on PE = out free-size (N) x cycles/row, independent of K and M.
So every conv matmul packs the full K=128 contraction:
  stage A (kv conv): K = 64 ch x 2 images (block-diagonal weights); image
    pair (2m, 2m+1) lives at SBUF partitions 0-63 / 64-127 at the same
    column offset.  9 taps x 16 pairs = 144 matmuls (vs 288 at K=64).
  stage B (score conv): K = 32 kv-ch x 4 images of one quad (kvb[g] is
    already [4 img x 32 ch, 441]); block-diag weights map img j, head h to
    out row 2j+h.  9 taps x 8 quads = 72 matmuls (vs 288 at K=32/64).
Scores land sparse (8 rows per 32-partition block, tile_position col must be
32-aligned); the exp() after the PE transpose compacts them via a strided AP.
"""

import sys

import ml_dtypes
import numpy as np

if "/opt/trn_rl_repo" not in sys.path:
    sys.path.insert(0, "/opt/trn_rl_repo")

import concourse.bass as bass
import concourse.mybir as mybir
import concourse.tile as tile
from concourse import bacc
from concourse.bass_utils import run_bass_kernel_spmd

# problem constants (hardcoded per contract)
B, T, HS, WS, C, NH = 4, 32, 19, 19, 64, 4
D = C // NH            # 16 per-head channels
PX = HS * WS           # 361 pixels
NCORE = 8
HPC = 2                # heads per core
M32 = HPC * D          # 32 kv channels per core
NQUAD = 8              # image quads (4 imgs each)
XCOLS = 16 * 361       # x_sb cols: 16 unpadded image-pair blocks
CHUNKS = [(0, 128), (128, 128), (256, 105)]  # pixel chunks (start, count)
# tap = 3*dy + dx; center tap first so every psum element is written before
# other taps accumulate onto it (per-element has_written semantics)
TAP_ORDER = [4, 0, 1, 2, 3, 5, 6, 7, 8]


def _tap_rects(tap):
    """Valid output rect and matching input offset for a SAME-pad conv tap."""
    dy, dx = tap // 3, tap % 3
    oy0, oy1 = max(0, 1 - dy), HS - max(0, dy - 1)
    ox0, ox1 = max(0, 1 - dx), WS - max(0, dx - 1)
    iy0, ix0 = oy0 + dy - 1, ox0 + dx - 1
    return oy0, ox0, oy1 - oy0, ox1 - ox0, iy0, ix0

F32 = mybir.dt.float32
BF16 = mybir.dt.bfloat16


def _kernel_body(tc, y, x_t, w_kv_t, w_s_t, b_vec, ident):
    nc = tc.nc

    from contextlib import ExitStack

    with ExitStack() as ctx:
        const = ctx.enter_context(tc.tile_pool(name="const", bufs=1))
        kvpool = ctx.enter_context(tc.tile_pool(name="kv", bufs=1))
        sbig = ctx.enter_context(tc.tile_pool(name="sbig", bufs=1))
        small = ctx.enter_context(tc.tile_pool(name="small", bufs=1))
        tmppool = ctx.enter_context(tc.tile_pool(name="tmp", bufs=2))
        psA = ctx.enter_context(tc.tile_pool(name="psA", bufs=2, space="PSUM"))
        psS = ctx.enter_context(tc.tile_pool(name="psS", bufs=1, space="PSUM"))
        psK = ctx.enter_context(tc.tile_pool(name="psK", bufs=2, space="PSUM"))
        psT = ctx.enter_context(tc.tile_pool(name="psT", bufs=2, space="PSUM"))

        # ---- load inputs: one packed bf16 const DMA (HWDGE costs ~625ns
        # per dma_start, serialized), then x in 3 chunks so stage A starts
        # as soon as quad 0 lands
        wpack_sb = const.tile([128, 777], BF16)
        nc.sync.dma_start(wpack_sb[:], w_kv_t[:])
        x_sb = const.tile([128, XCOLS], BF16)
        nc.sync.dma_start(x_sb[:, 0:722], x_t[:, 0:722])
        id_sb = const.tile([128, 128], F32)
        nc.sync.dma_start(id_sb[:], ident[:])
        nc.sync.dma_start(x_sb[:, 722 : 4 * 722], x_t[:, 722 : 4 * 722])
        nc.sync.dma_start(x_sb[:, 4 * 722 :], x_t[:, 4 * 722 :])
        # bias must be f32 for tensor_scalar ops: tiny on-device convert
        b32_sb = const.tile([128, 1], F32)
        nc.scalar.copy(b32_sb[:], wpack_sb[:, 648:649])
        # warm the Exp activation table off the critical path
        warm = const.tile([1, 1], F32)
        nc.gpsimd.memset(warm[:], 0.0)
        nc.scalar.activation(warm[:], warm[:],
                             mybir.ActivationFunctionType.Exp)
        # warm the PE p-state during the x DMA wait: pe_busy_start is set at
        # the first matmul, and full clock needs ~3us of ramp
        wz = const.tile([128, 256], BF16)
        nc.gpsimd.memset(wz[:], 0.0)
        for i in range(26):
            wps = psT.tile([128, 128], F32, tag="psTs", name=f"warm{i}")
            nc.tensor.matmul(wps[:], wz[:, 0:128], wz[:, 128:256])

        kv = [kvpool.tile([128, PX], BF16, tag=f"kv{g}", name=f"kv{g}")
              for g in range(NQUAD)]
        # padded (21x21) bf16 kv for stage-B windowed rhs reads
        kvb = [kvpool.tile([128, 441], BF16, tag=f"kvb{g}", name=f"kvb{g}")
               for g in range(NQUAD)]
        # scores, sparse rows: s_ps[half] partition 32*(g%4) + 2j + h
        s_ps = [psS.tile([128, PX], F32, tag=f"sps{hf}", name=f"sps{hf}")
                for hf in range(2)]
        s_sb = [sbig.tile([128, PX], F32, tag=f"ssb{hf}", name=f"ssb{hf}")
                for hf in range(2)]
        for hf in range(2):  # garbage rows flow through the PE transpose
            nc.gpsimd.memset(s_sb[hf][:], 0.0)
        # pixel-major kv: col = 1536*hf + 768*h + 256*c + 64*gq + 16*j + d
        kvT = sbig.tile([128, 3072], BF16)
        # exp(scores), pixel-major: col = 96*h + 48*hf + 16*c + 4*gq + j
        p_T = sbig.tile([128, 192], F32)

        def stage_a(g):
            ps = psA.tile([128, PX], F32, tag="psA", name=f"psA{g}")
            for ti, tap in enumerate(TAP_ORDER):
                oy0, ox0, nr, ncol, iy0, ix0 = _tap_rects(tap)
                lhs = wpack_sb[:, tap * 64 : (tap + 1) * 64]
                for pair in range(2):
                    m = 2 * g + pair
                    rhs = x_sb[:, m * PX : (m + 1) * PX].rearrange(
                        "p (a b) -> p a b", a=HS)[
                        :, iy0 : iy0 + nr, ix0 : ix0 + ncol]
                    out = ps[64 * pair : 64 * pair + 64, :].rearrange(
                        "p (a b) -> p a b", a=HS)[
                        :, oy0 : oy0 + nr, ox0 : ox0 + ncol]
                    nc.tensor.matmul(
                        out, lhs, rhs,
                        start=(ti == 0), stop=(ti == 8),
                        tile_position=(0, 64 * pair),
                        skip_group_check=True,
                    )
            return ps

        def stage_a_evac(g, ps):
            # evacuate: kv (+ per-channel bias) bf16, and bf16 padded copy.
            # Emitted AFTER stage_b(g-1) so the in-order DVE queue runs
            # compact(g-1) before this add (no head-of-line block).
            nc.vector.tensor_scalar_add(kv[g][:], ps[:], b32_sb[:])
            kvb_in = kvb[g][:].rearrange("p (a b) -> p a b", a=21)[:, 1:20, 1:20]
            kvsrc = kv[g][:].rearrange("p (a b) -> p a b", a=HS)
            if g == NQUAD - 1:
                nc.scalar.copy(kvb_in[:, 0:10, :], kvsrc[:, 0:10, :])
                nc.gpsimd.tensor_copy(kvb_in[:, 10:HS, :], kvsrc[:, 10:HS, :])
            else:
                nc.gpsimd.tensor_copy(kvb_in, kvsrc)

        def stage_b(g):
            hf, q = g // 4, g % 4
            for ti, tap in enumerate(TAP_ORDER):
                oy0, ox0, nr, ncol, iy0, ix0 = _tap_rects(tap)
                lhs = wpack_sb[:, 576 + tap * 8 : 576 + (tap + 1) * 8]
                rhs = kvb[g][:].rearrange("p (a b) -> p a b", a=21)[
                    :, 1 + iy0 : 1 + iy0 + nr, 1 + ix0 : 1 + ix0 + ncol]
                out = s_ps[hf][32 * q : 32 * q + 8, :].rearrange(
                    "p (a b) -> p a b", a=HS)[
                    :, oy0 : oy0 + nr, ox0 : ox0 + ncol]
                nc.tensor.matmul(
                    out, lhs, rhs,
                    start=(ti == 0), stop=(ti == 8),
                    tile_position=(0, 32 * q),
                    skip_group_check=True,
                )

        def stage_t(g):
            # kv[g] [128, px] -> pixel-major via PE transpose (bf16, 1 cyc/row)
            hf, gq = g // 4, g % 4
            ps = psK.tile([128, 384], BF16, tag="psK", name=f"psK{g}")
            for c, (p0, cnt) in enumerate(CHUNKS):
                nc.tensor.matmul(ps[0:cnt, 128 * c : 128 * c + 128],
                                 kv[g][:, p0 : p0 + cnt], wpack_sb[:, 649:777],
                                 is_transpose=True)
            for c, (p0, cnt) in enumerate(CHUNKS):
                # psK col = 32j + 16h + d  ->  kvT col 1536hf+768h+256c+64gq+16j+d
                src = ps[0:cnt, 128 * c : 128 * c + 128].rearrange(
                    "p (j h d) -> p j h d", j=4, h=2)
                ks = kvT[0:cnt, 1536 * hf + 256 * c + 64 * gq :]
                dst = bass.AP(tensor=ks.tensor, offset=ks.offset,
                              ap=[ks.ap[0], [16, 4], [768, 2], [1, D]])
                nc.scalar.copy(dst, src)

        def compact_half(hf):
            # three DVE pieces: each unblocks its transpose chunk asap
            nc.vector.tensor_copy(s_sb[hf][:, 0:128], s_ps[hf][:, 0:128])
            nc.vector.tensor_copy(s_sb[hf][:, 128:256], s_ps[hf][:, 128:256])
            nc.vector.tensor_copy(s_sb[hf][:, 256:PX], s_ps[hf][:, 256:PX])

        def stage_s(hf):
            # transpose this half's scores, exp-compact into p_T
            for c, (p0, cnt) in enumerate(CHUNKS):
                if hf == 1 and c == 2:
                    big = psA.tile([128, PX], F32, tag="psA", name=f"psTs{c}{hf}")
                    ps_t = big[:, 0:128]
                else:
                    ps_t = psT.tile([128, 128], F32, tag="psTs",
                                    name=f"psTs{c}{hf}")
                nc.tensor.matmul(ps_t[0:cnt, :], s_sb[hf][:, p0 : p0 + cnt],
                                 id_sb, is_transpose=True)
                # ps_t col = 32q + 2j + h (cols 8-31 of each 32-block unused)
                pss = ps_t[0:cnt, :]
                src = bass.AP(tensor=pss.tensor, offset=pss.offset,
                              ap=[pss.ap[0], [32, 4], [2, 4], [1, 2]])
                pts = p_T[0:cnt, 48 * hf + 16 * c :]
                dst = bass.AP(tensor=pts.tensor, offset=pts.offset,
                              ap=[pts.ap[0], [4, 4], [1, 4], [96, 2]])
                nc.scalar.activation(dst, src, mybir.ActivationFunctionType.Exp)

        z4 = small.tile([128, 12], F32)    # cols: 6*hf + 3*h + c
        z2 = small.tile([128, 6], F32)     # cols: 3*h + c
        zi = small.tile([128, 6], F32)
        acc = small.tile([128, 192], F32)  # cols: 96*hf + 32*c + 16*h + d

        def _zchain(hf):
            # z[p, h, c] = sum over r=(gq,j) of exp-scores for this half
            pz = p_T[:, 48 * hf : 48 * hf + 48]
            zin = bass.AP(tensor=pz.tensor, offset=pz.offset,
                          ap=[pz.ap[0], [96, 2], [16, 3], [1, 16]])
            nc.vector.reduce_sum(
                z4[:, 6 * hf : 6 * hf + 6].rearrange("p (h c) -> p h c", h=2),
                zin, axis=mybir.AxisListType.X)
            if hf == 1:
                nc.vector.tensor_add(z2[:], z4[:, 0:6], z4[:, 6:12])
                nc.vector.reciprocal(zi[:], z2[:])

        def _reduce_c(hf, t, c):
            tin = t[:, 256 * c : 256 * c + 256]
            rin = bass.AP(tensor=tin.tensor, offset=tin.offset,
                          ap=[tin.ap[0], [768, 2], [1, D], [16, 16]])
            nc.vector.reduce_sum(
                acc[:, 96 * hf + 32 * c : 96 * hf + 32 * c + 32].rearrange(
                    "p (h d) -> p h d", h=2),
                rin, axis=mybir.AxisListType.X)

        def stage_d_half(hf):
            # per chunk: 2 mults (DVE+Pool) then the reduce, so the DVE
            # starts right after the first exp; z-chain last
            t = small.tile([128, 1536], F32, tag=f"tD{hf}", name=f"tD{hf}")
            for c in range(3):
                for h in range(HPC):
                    base = 1536 * hf + 768 * h + 256 * c
                    v0 = kvT[:, base : base + 256].rearrange(
                        "p (gj d) -> p gj d", d=D)
                    pv = p_T[:, 96 * h + 48 * hf + 16 * c :]
                    v1 = bass.AP(tensor=pv.tensor, offset=pv.offset,
                                 ap=[pv.ap[0], [1, 16], [0, D]])
                    eng = nc.vector if h == 0 else nc.gpsimd
                    eng.tensor_mul(
                        t[:, 768 * h + 256 * c : 768 * h + 256 * c + 256
                          ].rearrange("p (gj d) -> p gj d", d=D), v0, v1)
                _reduce_c(hf, t, c)
            _zchain(hf)
            if hf == 1:
                accS = small.tile([128, 96], F32)  # cols: 32*c + 16*h + d
                nc.vector.tensor_add(accS[:], acc[:, 0:96], acc[:, 96:192])
                outT = small.tile([128, 96], F32)
                v0 = accS[:].rearrange("p (c h d) -> p c h d", c=3, h=2)
                v1 = bass.AP(tensor=zi.tensor, offset=zi[:].offset,
                             ap=[zi[:].ap[0], [1, 3], [3, 2], [0, D]])
                nc.vector.tensor_mul(
                    outT[:].rearrange("p (c h d) -> p c h d", c=3, h=2),
                    v0, v1)
                nc.sync.dma_start(y[:], outT[:])

        def stage_d_tail():
            pass

        # software pipeline: PE order A0, A1, [T0 B0], A2, [T1 B1], ...
        pend = None
        for g in range(NQUAD):
            ps = stage_a(g)
            if g >= 1:
                stage_t(g - 1)
                stage_b(g - 1)
            if g >= 1 and g - 1 == 3:
                compact_half(0)
            stage_a_evac(g, ps)
            if g >= 1 and g - 1 == 4:
                stage_s(0)
                stage_d_half(0)
        stage_t(NQUAD - 1)
        stage_b(NQUAD - 1)
        compact_half(1)
        stage_s(1)
        stage_d_half(1)
        stage_d_tail()


_CACHE = {}


def _build_program():
    if "nc" in _CACHE:
        return _CACHE["nc"]
    nc = bacc.Bacc("TRN2", target_bir_lowering=False, debug=False,
                   num_devices=NCORE)
    x_t = nc.dram_tensor("x_t", [128, XCOLS], BF16, kind="ExternalInput").ap()
    w_kv_t = nc.dram_tensor("w_kv_t", [128, 777], BF16,
                            kind="ExternalInput").ap()
    ident = nc.dram_tensor("ident", [128, 128], F32, kind="ExternalInput").ap()
    y = nc.dram_tensor("y", [128, 96], F32, kind="ExternalOutput").ap()
    with tile.TileContext(nc) as tc:
        _kernel_body(tc, y, x_t, w_kv_t, None, None, ident)
    nc.compile()
    _CACHE["nc"] = nc
    return nc


def make_in_maps(x, w_kv, b_kv, w_att):
    """Host-side shard prep: per-core input dicts."""
    x = np.asarray(x, np.float32)
    w_kv = np.asarray(w_kv, np.float32)
    b_kv = np.asarray(b_kv, np.float32)
    w_att = np.asarray(w_att, np.float32)
    ident = np.eye(128, dtype=np.float32)
    in_maps = []
    # x_sb layout: col block m (441 = padded 21x21) holds img 2m on
    # partitions 0-63 (its 64 channels) and img 2m+1 on partitions 64-127.
    xt_all = []
    for b in range(B):
        xp = np.zeros((2, 64, 16, HS, WS), np.float32)
        for m in range(16):
            xp[0, :, m] = x[b, 2 * m].transpose(2, 0, 1)
            xp[1, :, m] = x[b, 2 * m + 1].transpose(2, 0, 1)
        xt_all.append(xp.reshape(128, XCOLS).astype(ml_dtypes.bfloat16))
    for core in range(NCORE):
        b, hb = core // 2, (core % 2) * HPC
        # stage A weights: per tap [128, 64] block-diagonal over the img pair
        wk = np.zeros((128, 9, 64), np.float32)
        for tap in range(9):
            dy, dx = tap // 3, tap % 3
            for hh in range(HPC):
                blk = w_kv[hb + hh, dy, dx]          # [64, 16]
                wk[0:64, tap, 16 * hh : 16 * hh + 16] = blk
                wk[64:128, tap, 32 + 16 * hh : 32 + 16 * hh + 16] = blk
        # stage B weights: per tap [128, 8]; row 32j+16h+d -> out col 2j+h
        ws = np.zeros((128, 9, 8), np.float32)
        for tap in range(9):
            dy, dx = tap // 3, tap % 3
            for j in range(4):
                for hh in range(HPC):
                    ws[32 * j + 16 * hh : 32 * j + 16 * hh + 16, tap,
                       2 * j + hh] = w_att[hb + hh, dy, dx, :D, 0]
        bv = np.tile(np.concatenate([b_kv[hb], b_kv[hb + 1]]), 4)
        wpack = np.concatenate(
            [wk.reshape(128, 9 * 64), ws.reshape(128, 9 * 8),
             bv.reshape(128, 1), ident], axis=1)
        in_maps.append({"x_t": xt_all[b],
                        "w_kv_t": wpack.astype(ml_dtypes.bfloat16),
                        "ident": ident})
    return in_maps


def assemble(results):
    out = np.empty((B, T, HS, WS, C), np.float32)
    for core in range(NCORE):
        b, hb = core // 2, (core % 2) * M32
        yc = np.asarray(results[core]["y"])  # [128, 32c + 16h + d]
        ypx = np.empty((PX, M32), np.float32)
        for c, (p0, cnt) in enumerate(CHUNKS):
            ypx[p0 : p0 + cnt, :] = yc[0:cnt, 32 * c : 32 * c + 32]
        out[b, :, :, :, hb : hb + M32] = ypx.reshape(HS, WS, M32)[None]
    return out


def kernel(x, w_q, b_q, w_kv, b_kv, w_att, b_att, **_unused):
    nc = _build_program()
    in_maps = make_in_maps(x, w_kv, b_kv, w_att)
    res = run_bass_kernel_spmd(nc, in_maps, core_ids=list(range(NCORE)))
    return assemble(res.results)


if __name__ == "__main__":
    rng = np.random.default_rng(0)
    ins = {
        "x": rng.standard_normal((B, T, HS, WS, C)).astype(np.float32),
        "w_q": rng.standard_normal((NH, 3, 3, C, D)).astype(np.float32) * 0.05,
        "b_q": np.zeros((NH, D), np.float32),
        "w_kv": rng.standard_normal((NH, 3, 3, C, D)).astype(np.float32) * 0.05,
        "b_kv": np.zeros((NH, D), np.float32),
        "w_att": rng.standard_normal((NH, 3, 3, 2 * D, 1)).astype(np.float32) * 0.05,
        "b_att": np.zeros((NH, 1), np.float32),
    }
    out = kernel(**ins)
    print("kernel output", out.shape, out.dtype)
